# revision 1
# baseline (speedup 1.0000x reference)
"""Trainium2 Bass kernel for nn_MultiHeadAttention_446676599023.

Strategy (8 NeuronCores, SPMD, no collectives):
  core c -> batch b = c//2, head-group g = c%2 (heads 8g..8g+7, E-dims 512g..512g+512).

Math: reference computes attn_out = softmax(QK^T/sqrt(D)) @ V per head, projects with
Wo, takes mean over sequence, normalizes, subtracts text_array, then a tiny MLP.
mean_S commutes with the output projection, so each core only needs
  r_h[d] = sum_q softmax_row(q) @ V_h  summed over q   (shape [64] per head)
and the whole Wo/normalize/MLP tail runs on host on a [4,1024] tensor (exact algebra,
negligible FLOPs). Device work per core:
  - Q^T,K^T = (Wq x^T), [d-part, seq-free] layout; V = x Wv^T in [seq-part, d-free].
  - scores^T[k,q] = K^T(d,k)^T-free matmul: lhsT=K^T slice, rhs=Q^T slice (contraction d=64;
    even/odd heads land on PE row-groups 0/64 -> concurrent 2-head packing).
  - E = exp(scores/8 + maskbias_k) on ScalarE straight out of PSUM (no row-max needed:
    scores ~ N(0,1) so exp never overflows; mask folds into the per-partition bias).
  - P^T[d,q] (+ Z row) = matmul(lhsT=V_aug[k,65], rhs=E^T[k,q]) accumulated over k-tiles,
    where V_aug has a ones column -> row 64 of P^T is the softmax denominator Z.
  - finalize: w = 1/Z, broadcast w via a DRAM bounce DMA, r = sum_q P^T * w (DVE fused
    multiply-reduce). Output res[h,d] per core; host divides by S and applies the tail.
All matmuls run as float32r (full-rate fp32 on the PE for free-dim >= 256).
"""

import math
import os
import sys

import numpy as np

for _p in ("/opt/trn_rl_repo",):
    if _p not in sys.path and os.path.isdir(_p):
        sys.path.append(_p)

B, S, E, H = 4, 2048, 1024, 16
D = E // H            # 64 head dim
G = 2                 # head groups (tensor-parallel factor)
EG = E // G           # 512 dims per group
HG = H // G           # 8 heads per group
NCORES = 8
PART = 128
ET = E // PART        # 8 contraction tiles for projections
KT = S // PART        # 16 key tiles
MT = EG // PART       # 4 m-tiles (= head pairs) per group
QH = 2                # q halves
QHW = S // QH         # 1024
NEG = -1.0e30

_CACHE: dict = {}


def _build(repeat: int = 1):
    """Build the Bacc module (one SPMD program, same on all 8 cores)."""
    ablate = os.environ.get("BASS_KERNEL_ABLATE", "")
    import concourse.bacc as bacc
    import concourse.mybir as mybir
    import concourse.tile as tile
    from contextlib import ExitStack

    f32 = mybir.dt.float32
    f32r = mybir.dt.float32r
    AF = mybir.ActivationFunctionType
    AX = mybir.AxisListType

    nc = bacc.Bacc("TRN2", target_bir_lowering=False, debug=False)
    xT = nc.dram_tensor("xT", [E, S], f32r, kind="ExternalInput").ap()
    wqT = nc.dram_tensor("wqT", [E, EG], f32r, kind="ExternalInput").ap()
    wkT = nc.dram_tensor("wkT", [E, EG], f32r, kind="ExternalInput").ap()
    wvT = nc.dram_tensor("wvT", [E, EG], f32r, kind="ExternalInput").ap()
    mbT = nc.dram_tensor("mbT", [PART, KT], f32, kind="ExternalInput").ap()
    bqT = nc.dram_tensor("bqT", [PART, MT], f32, kind="ExternalInput").ap()
    bkT = nc.dram_tensor("bkT", [PART, MT], f32, kind="ExternalInput").ap()
    res = nc.dram_tensor("res", [repeat, HG, D], f32, kind="ExternalOutput").ap()

    QC = 4          # q chunks of 512
    QW = S // QC    # 512

    with tile.TileContext(nc) as tc, ExitStack() as ctx:
        const_p = ctx.enter_context(tc.tile_pool(name="const", bufs=1))
        xt_p = ctx.enter_context(tc.tile_pool(name="xt", bufs=ET))
        wv_p = ctx.enter_context(tc.tile_pool(name="wv", bufs=ET))
        wqk_p = ctx.enter_context(tc.tile_pool(name="wqk", bufs=10))
        qt_p = ctx.enter_context(tc.tile_pool(name="qt", bufs=2))
        kt_p = ctx.enter_context(tc.tile_pool(name="kt", bufs=2))
        v_p = ctx.enter_context(tc.tile_pool(name="v", bufs=KT))
        et_p = ctx.enter_context(tc.tile_pool(name="et", bufs=4))
        psb_p = ctx.enter_context(tc.tile_pool(name="psb", bufs=2))
        wrep_p = ctx.enter_context(tc.tile_pool(name="wrep", bufs=2))
        fin_p = ctx.enter_context(tc.tile_pool(name="fin", bufs=2))
        # PSUM: "sc" = score tiles [128,1024] (2 banks) x2; "pp" = 1-bank
        # accumulators (V/QK projection chunks + E@V pacc) x4  -> 8 banks total
        sc_ps = ctx.enter_context(tc.tile_pool(name="scps", bufs=2, space="PSUM"))
        p_ps = ctx.enter_context(tc.tile_pool(name="pps", bufs=3, space="PSUM"))
        qk_ps = ctx.enter_context(tc.tile_pool(name="qkps", bufs=1, space="PSUM"))
        wd_p = ctx.enter_context(tc.tile_pool(name="wd", bufs=2, space="DRAM"))

        for rep in range(repeat):
            mb = const_p.tile([PART, KT], f32, tag="mb")
            nc.sync.dma_start(mb[:], mbT[:])
            bq = const_p.tile([PART, MT], f32, tag="bq")
            nc.sync.dma_start(bq[:], bqT[:])
            bk = const_p.tile([PART, MT], f32, tag="bk")
            nc.sync.dma_start(bk[:], bkT[:])
            ones_c = const_p.tile([PART, HG], f32, tag="ones_c")
            nc.vector.memset(ones_c[:], 1.0)

            # interleave x / Wv loads so the first V-proj matmul can start after
            # the first (xt, wv) pair lands instead of after the whole 14MB
            xt = []
            wv = []
            for i in range(ET):
                t = xt_p.tile([PART, S], f32r, tag="xt")
                nc.sync.dma_start(t[:], xT[i * PART : (i + 1) * PART, :])
                xt.append(t)
                t = wv_p.tile([PART, EG], f32r, tag="wv")
                nc.sync.dma_start(t[:], wvT[i * PART : (i + 1) * PART, :])
                wv.append(t)

            # ---- V projection: V[k, e'] with per-head ones column (65-stride) ----
            v_sb = []
            for ks in range(KT):
                vt = v_p.tile([PART, HG * 65], f32r, tag="v")
                v3 = vt[:].rearrange("p (h c) -> p h c", c=65)
                nc.vector.tensor_copy(
                    v3[:, :, 64:65], ones_c[:].rearrange("p (h o) -> p h o", o=1)
                )
                if ablate == "noqkv":
                    nc.vector.tensor_copy(
                        v3[:, :, 0:64],
                        xt[0][:, 0:EG].rearrange("p (h c) -> p h c", c=64),
                    )
                else:
                    ps = qk_ps.tile([PART, EG], f32, tag="qkp", name="vps")
                    for i in range(ET):
                        nc.tensor.matmul(
                            ps[:],
                            lhsT=xt[i][:, ks * PART : (ks + 1) * PART],
                            rhs=wv[i][:],
                            start=(i == 0),
                            stop=(i == ET - 1),
                        )
                    nc.vector.tensor_copy(
                        v3[:, :, 0:64],
                        ps[:].rearrange("p (h c) -> p h c", c=64),
                    )
                v_sb.append(vt)

            # ---- per head-pair p: Q/K projection for m-tile p, then attention ----
            for p in range(MT):
                qt_m = qt_p.tile([PART, S], f32r, tag="qt")
                kt_m = kt_p.tile([PART, S], f32r, tag="kt")
                if ablate == "noqkv":
                    nc.vector.tensor_copy(qt_m[:], xt[0][:])
                    nc.vector.tensor_copy(kt_m[:], xt[1][:])
                for dst, wT, bias in () if ablate == "noqkv" else ((qt_m, wqT, bq), (kt_m, wkT, bk)):
                    wtiles = []
                    for i in range(ET):
                        t = wqk_p.tile([PART, PART], f32r, tag="wqk")
                        nc.sync.dma_start(
                            t[:],
                            wT[i * PART : (i + 1) * PART, p * PART : (p + 1) * PART],
                        )
                        wtiles.append(t)
                    for qc in range(QC):
                        ps = qk_ps.tile([PART, QW], f32, tag="qkp", name="qkps")
                        for i in range(ET):
                            nc.tensor.matmul(
                                ps[:],
                                lhsT=wtiles[i][:],
                                rhs=xt[i][:, qc * QW : (qc + 1) * QW],
                                start=(i == 0),
                                stop=(i == ET - 1),
                            )
                        nc.vector.tensor_scalar_add(
                            dst[:, qc * QW : (qc + 1) * QW],
                            ps[:],
                            bias[:, p : p + 1],
                        )

                # attention: heads A=2p (PE rows 0:64) and B=2p+1 (rows 64:128)
                p_sb = {}
                racc = {}
                for hl in (0, 1):
                    p_sb[hl] = psb_p.tile([65, S], f32, tag="psb", name=f"psb{hl}")
                    racc[hl] = fin_p.tile([64, QC], f32, tag="racc", name=f"racc{hl}")
                for qc in range(QC):
                    pacc = {}
                    for hl in (0, 1):
                        pacc[hl] = p_ps.tile(
                            [65, QW], f32, tag="pp", name=f"pacc{hl}"
                        )
                    for kt in range(KT):
                        # one [128,1024] score tile: head A in cols 0:512,
                        # head B in 512:1024; the two matmuls hit PE
                        # row-groups 0/64 -> run concurrently
                        scps = sc_ps.tile([PART, 2 * QW], f32, tag="sc")
                        for hl in (0, 1):
                            r0 = hl * 64
                            nc.tensor.matmul(
                                scps[:, hl * QW : (hl + 1) * QW],
                                lhsT=kt_m[r0 : r0 + 64, kt * PART : (kt + 1) * PART],
                                rhs=qt_m[r0 : r0 + 64, qc * QW : (qc + 1) * QW],
                            )
                        e = et_p.tile([PART, 2 * QW], f32r, tag="et")
                        nc.scalar.activation(
                            e[:],
                            scps[:],
                            AF.Exp,
                            bias=mb[:, kt : kt + 1],
                            scale=1.0 / math.sqrt(D),
                        )
                        for hl in (0, 1):
                            nc.tensor.matmul(
                                pacc[hl][:],
                                lhsT=v_sb[kt][:, 65 * (2 * p + hl) : 65 * (2 * p + hl) + 65],
                                rhs=e[:, hl * QW : (hl + 1) * QW],
                                start=(kt == 0),
                                stop=(kt == KT - 1),
                            )
                    # drain + pipelined finalize for this q-chunk
                    for hl in (0, 1):
                        sl = slice(qc * QW, (qc + 1) * QW)
                        nc.vector.tensor_copy(p_sb[hl][:, sl], pacc[hl][:])
                        if ablate == "nofin":
                            nc.vector.reduce_sum(
                                out=racc[hl][:, qc : qc + 1],
                                in_=p_sb[hl][0:64, sl],
                                axis=AX.X,
                            )
                            continue
                        nc.vector.reciprocal(p_sb[hl][64:65, sl], p_sb[hl][64:65, sl])
                        wd = wd_p.tile([1, QW], f32, tag="wd")
                        nc.sync.dma_start(wd[:], p_sb[hl][64:65, sl])
                        wrep = wrep_p.tile([64, QW], f32, tag="wrep")
                        nc.sync.dma_start(wrep[:], wd[:].broadcast_to([64, QW]))
                        nc.vector.tensor_mul(wrep[:], p_sb[hl][0:64, sl], wrep[:])
                        nc.vector.reduce_sum(
                            out=racc[hl][:, qc : qc + 1], in_=wrep[:], axis=AX.X
                        )

                for hl in (0, 1):
                    r = fin_p.tile([64, 1], f32, tag="r")
                    nc.vector.reduce_sum(out=r[:], in_=racc[hl][:], axis=AX.X)
                    nc.sync.dma_start(res[rep, 2 * p + hl, :], r[:])

    nc.compile()
    return nc


def get_nc(repeat: int = 1):
    key = ("nc", repeat, os.environ.get("BASS_KERNEL_ABLATE", ""))
    if key not in _CACHE:
        _CACHE[key] = _build(repeat)
    return _CACHE[key]


def make_in_maps(x, mask, Wq, bq, Wk, bk, Wv):
    """Per-core input dict (core c -> batch c//2, head-group c%2)."""
    x = np.asarray(x, np.float32)
    mask = np.asarray(mask)
    maskbias = (mask == 0).astype(np.float32) * NEG  # [B, S]
    in_maps = []
    xTb = [np.ascontiguousarray(x[b].T) for b in range(B)]
    mbTb = [np.ascontiguousarray(maskbias[b].reshape(KT, PART).T) for b in range(B)]
    slabs = {}
    for g in range(G):
        sl = slice(g * EG, (g + 1) * EG)
        slabs[g] = (
            np.ascontiguousarray(np.asarray(Wq, np.float32)[sl].T),
            np.ascontiguousarray(np.asarray(Wk, np.float32)[sl].T),
            np.ascontiguousarray(np.asarray(Wv, np.float32)[sl].T),
            np.ascontiguousarray(np.asarray(bq, np.float32)[sl].reshape(MT, PART).T),
            np.ascontiguousarray(np.asarray(bk, np.float32)[sl].reshape(MT, PART).T),
        )
    for c in range(NCORES):
        b, g = c // G, c % G
        wq_t, wk_t, wv_t, bq_t, bk_t = slabs[g]
        in_maps.append(
            {
                "xT": xTb[b],
                "wqT": wq_t,
                "wkT": wk_t,
                "wvT": wv_t,
                "mbT": mbTb[b],
                "bqT": bq_t,
                "bkT": bk_t,
            }
        )
    return in_maps


def host_tail(mean_attn, text_array, bv, Wo, bo, W1, b1, W2, b2):
    """Exact tail on [B, E]: out_proj (after the mean), normalize, sub, MLP."""
    out = mean_attn + np.asarray(bv, np.float32)[None, :]
    out = out @ np.asarray(Wo, np.float32).T + np.asarray(bo, np.float32)
    out = out / np.linalg.norm(out, axis=-1, keepdims=True)
    out = out - np.asarray(text_array, np.float32)
    h = np.maximum(out @ np.asarray(W1, np.float32).T + np.asarray(b1, np.float32), 0.0)
    return np.tanh(h @ np.asarray(W2, np.float32).T + np.asarray(b2, np.float32))


def kernel(
    x, mask, text_array, Wq, bq, Wk, bk, Wv, bv, Wo, bo, W1, b1, W2, b2
):
    from concourse.bass_utils import run_bass_kernel_spmd

    nc = get_nc()
    in_maps = make_in_maps(x, mask, Wq, bq, Wk, bk, Wv)
    out = run_bass_kernel_spmd(nc, in_maps, core_ids=list(range(NCORES)))
    mean_attn = np.zeros((B, E), np.float32)
    for c in range(NCORES):
        b, g = c // G, c % G
        r = out.results[c]["res"][0]  # [HG, D], sum_q attn_out; divide by S below
        mean_attn[b, g * EG : (g + 1) * EG] = r.reshape(EG) / S
    return host_tail(mean_attn, text_array, bv, Wo, bo, W1, b1, W2, b2).astype(
        np.float32
    )



# revision 12
# speedup vs baseline: 1.3245x; 1.3245x over previous
"""Trainium2 Bass kernel for nn_MultiHeadAttention_446676599023 (v2, fp8).

Strategy (8 NeuronCores, SPMD, no collectives):
  core c -> batch b = c//2, head-group g = c%2 (heads 8g..8g+7, E-dims 512g..512g+512).

The mean over S commutes with the output projection, so each core returns only
  r_h[d] = sum_q softmax_row(q) @ V_h   (shape [64] per head)
and the Wo/normalize/MLP tail runs on the host on a [4,1024] tensor (exact algebra).

Device pipeline per core (all matmul operands fp8):
  - Projections in fp8e4m3 with DoubleRow (contraction pairs packed along the
    free dim): Q/K -> [128, S] per head-pair (2 heads on PE row-groups 0/64),
    V -> [keys, e'] with a per-head ones column for the softmax denominator.
    Weights are pre-scaled by 32 on the host so fp8 sees ~N(0,1) values.
  - Scores: fp8e4m3 DoubleRow matmuls with K=32 contraction pairs. Each head
    owns one 32-partition quadrant of two [128, 2, S] Q/K tiles (host permutes
    W rows so the projection lands directly in this layout); tile_position
    places the matmul in the head's PE quadrant. s_psum = 1024 * (q . k), f32.
    (Plain fp8 K=64 matmuls packed on row-groups 0/64 hang the device - FWL
    weight loads racing the other row-group's in-flight matmul; DoubleRow
    disables FWL, so this path is also the workaround.)
  - exp: split between ScalarE (true exp, output fp8e5m2) and VectorE using a
    Schraudolph bit-trick: u8 = s_psum*(log2e/2048) + 60.5 truncated to uint8
    IS the fp8e5m2 encoding of exp(s_psum/8192) (log-linear mantissa approx,
    ~5% per-element error that cancels in softmax normalization).
  - E@V: fp8 DoubleRow over key-tile pairs (lhsT = V_aug pairs [128,2,65],
    rhs = E pairs [128,2,512]), accumulated in PSUM; row 64 is the softmax
    denominator Z via the ones column.
  - finalize: Z -> DRAM bounce -> [128,16] -> reciprocal (DVE) -> DRAM ->
    broadcast wrep [64,2048]; GpSimd fused multiply+reduce gives
    r = sum_q P[:,q] * w[q]. Host divides by 32*S and applies the tail.
"""

import math
import os
import sys

import numpy as np

for _p in ("/opt/trn_rl_repo",):
    if _p not in sys.path and os.path.isdir(_p):
        sys.path.append(_p)

B, S, E, H = 4, 2048, 1024, 16
D = E // H            # 64 head dim
G = 2                 # head groups (tensor-parallel factor)
EG = E // G           # 512 dims per group
HG = H // G           # 8 heads per group
NCORES = 8
PART = 128
ET = E // PART        # 8 contraction tiles
NPAIR = ET // 2       # 4 contraction pairs for DoubleRow
KT = S // PART        # 16 key tiles
NJ = KT // 2          # 8 key-tile pairs
MT = EG // PART       # 4 m-tiles (= head pairs) per group
QC = 4                # q chunks
QW = S // QC          # 512
NEG = -1.0e30
WSCALE = 32.0         # host-side weight prescale for fp8
LOG2E = 1.4426950408889634
# of every 16 exp tiles, this many go to ScalarE (rest: DVE bit-trick)
ACT_OF_16 = 9

_CACHE: dict = {}


def _build(repeat: int = 1, use_dve_exp: bool = True):
    import concourse.bacc as bacc
    import concourse.mybir as mybir
    import concourse.tile as tile
    from contextlib import ExitStack

    f32 = mybir.dt.float32
    f8e4 = mybir.dt.float8e4
    f8e5 = mybir.dt.float8e5
    u8 = mybir.dt.uint8
    AF = mybir.ActivationFunctionType
    ALU = mybir.AluOpType
    DR = mybir.MatmulPerfMode.DoubleRow

    nc = bacc.Bacc("TRN2", target_bir_lowering=False, debug=False)
    xp = nc.dram_tensor("xp", [PART, ET, S], f8e4, kind="ExternalInput").ap()
    wqp = nc.dram_tensor("wqp", [PART, ET, EG], f8e4, kind="ExternalInput").ap()
    wkp = nc.dram_tensor("wkp", [PART, ET, EG], f8e4, kind="ExternalInput").ap()
    wvp = nc.dram_tensor("wvp", [PART, ET, EG], f8e4, kind="ExternalInput").ap()
    bq2 = nc.dram_tensor("bq2", [PART, MT], f32, kind="ExternalInput").ap()
    bk2 = nc.dram_tensor("bk2", [PART, MT], f32, kind="ExternalInput").ap()
    mbb = nc.dram_tensor("mbb", [PART, KT], f32, kind="ExternalInput").ap()
    res = nc.dram_tensor("res", [repeat, HG, D], f32, kind="ExternalOutput").ap()

    C0 = LOG2E / 2048.0  # DVE trick scale on s_psum

    with tile.TileContext(nc) as tc, ExitStack() as ctx:
        const_p = ctx.enter_context(tc.tile_pool(name="const", bufs=1))
        xp_p = ctx.enter_context(tc.tile_pool(name="xp", bufs=1))
        w_p = ctx.enter_context(tc.tile_pool(name="w", bufs=3))
        qk_p = ctx.enter_context(tc.tile_pool(name="qk", bufs=2 * MT))
        v_p = ctx.enter_context(tc.tile_pool(name="v", bufs=NJ))
        e_p = ctx.enter_context(tc.tile_pool(name="e", bufs=3))
        psb_p = ctx.enter_context(tc.tile_pool(name="psb", bufs=4))
        zw_p = ctx.enter_context(tc.tile_pool(name="zw", bufs=2))
        wrep_p = ctx.enter_context(tc.tile_pool(name="wrep", bufs=2))
        wtmp_p = ctx.enter_context(tc.tile_pool(name="wtmp", bufs=2))
        fin_p = ctx.enter_context(tc.tile_pool(name="fin", bufs=4))
        # PSUM: scores 3 x [128,1024] (2 banks each) + pacc 2 x [65,512]
        sc_ps = ctx.enter_context(tc.tile_pool(name="scps", bufs=3, space="PSUM"))
        p_ps = ctx.enter_context(tc.tile_pool(name="pps", bufs=2, space="PSUM"))
        wd_p = ctx.enter_context(tc.tile_pool(name="wd", bufs=4, space="DRAM"))

        for rep in range(repeat):
            mb = const_p.tile([PART, KT], f32, tag="mb")
            nc.sync.dma_start(mb[:], mbb[:])
            bq_t = const_p.tile([PART, MT], f32, tag="bq")
            nc.sync.dma_start(bq_t[:], bq2[:])
            bk_t = const_p.tile([PART, MT], f32, tag="bk")
            nc.sync.dma_start(bk_t[:], bk2[:])

            x_t = xp_p.tile([PART, ET, S], f8e4, tag="x")
            nc.sync.dma_start(x_t[:], xp[:])
            wq_t = w_p.tile([PART, ET, EG], f8e4, tag="w", name="wq")
            nc.sync.dma_start(wq_t[:], wqp[:])
            wk_t = w_p.tile([PART, ET, EG], f8e4, tag="w", name="wk")
            nc.sync.dma_start(wk_t[:], wkp[:])
            wv_t = w_p.tile([PART, ET, EG], f8e4, tag="w", name="wv")
            nc.sync.dma_start(wv_t[:], wvp[:])

            # ---------------- projections (fp8 DoubleRow) ----------------
            # Q/K land in quadrant layout: tile t in {0,1} is [128, 2, S];
            # head h = 4t + p//32 lives on partitions 32(h%4):32(h%4)+32 with
            # d = i*32 + p%32 (host permutes W rows/bias to match).
            qk = {}
            n_conv = 0
            for which in ("q", "k"):
                for t_ in range(2):
                    qk[(which, t_)] = qk_p.tile(
                        [PART, 2, S], f8e4, tag="qk", name=f"{which}{t_}"
                    )
            for c in range(MT):
                t_, i_ = c // 2, c % 2
                for which, w_t, b_t in (("q", wq_t, bq_t), ("k", wk_t, bk_t)):
                    t = qk[(which, t_)]
                    for qc in range(QC):
                        ps = sc_ps.tile([PART, 2 * QW], f32, tag="sc", name="qkps")
                        for i in range(NPAIR):
                            nc.tensor.matmul(
                                ps[:, 0:QW],
                                lhsT=w_t[:, 2 * i : 2 * i + 2, c * PART : (c + 1) * PART],
                                rhs=x_t[:, 2 * i : 2 * i + 2, qc * QW : (qc + 1) * QW],
                                start=(i == 0),
                                stop=(i == NPAIR - 1),
                                perf_mode=DR,
                            )
                        # psum -> fp8 with bias, alternating ACT/DVE
                        if n_conv % 2 == 0:
                            nc.vector.tensor_scalar_add(
                                t[:, i_, qc * QW : (qc + 1) * QW],
                                ps[:, 0:QW],
                                b_t[:, c : c + 1],
                            )
                        else:
                            nc.scalar.add(
                                t[:, i_, qc * QW : (qc + 1) * QW],
                                ps[:, 0:QW],
                                b_t[:, c : c + 1],
                            )
                        n_conv += 1

            # ---------------- V projection (fp8 DoubleRow) ----------------
            v_sb = []
            for j in range(NJ):
                vt = v_p.tile([PART, 2, HG * 68], f8e4, tag="v")
                v4 = vt[:].rearrange("p i (h c) -> p i h c", c=68)
                nc.gpsimd.memset(v4[:, :, :, 64:68], 1.0)
                v_sb.append(vt)
            for ks in range(KT):
                ps = sc_ps.tile([PART, 2 * QW], f32, tag="sc", name="vps")
                for i in range(NPAIR):
                    nc.tensor.matmul(
                        ps[:, 0:QW],
                        lhsT=x_t[:, 2 * i : 2 * i + 2, ks * PART : (ks + 1) * PART],
                        rhs=wv_t[:, 2 * i : 2 * i + 2, :],
                        start=(i == 0),
                        stop=(i == NPAIR - 1),
                        perf_mode=DR,
                    )
                v4 = v_sb[ks // 2][:].rearrange("p i (h c) -> p i h c", c=68)
                dst = v4[:, ks % 2, :, 0:64]
                src = ps[:, 0:QW].rearrange("p (h c) -> p h c", c=64)
                if ks % 2 == 0:
                    nc.vector.tensor_copy(dst, src)
                else:
                    nc.scalar.copy(dst, src)

            # ---------------- attention ----------------
            n_exp = 0
            for m in range(MT):
                psb = {}
                for hl in (0, 1):
                    psb[hl] = psb_p.tile([65, S], f32, tag="psb", name=f"psb{m}_{hl}")
                for qc in range(QC):
                    pacc = {}
                    for hl in (0, 1):
                        pacc[hl] = p_ps.tile([65, QW], f32, tag="pp", name=f"pacc{hl}")
                    for j in range(NJ):
                        et = e_p.tile([PART, 2, 2, QW], f8e5, tag="e")
                        for hl in (0, 1):
                            h = 2 * m + hl
                            t_, qd = h // 4, h % 4
                            qt = qk[("q", t_)]
                            kt_ = qk[("k", t_)]
                            r0 = 32 * qd
                            scps = sc_ps.tile([PART, 2 * QW], f32, tag="sc", name="scps")
                            for i in (0, 1):
                                kt = 2 * j + i
                                nc.tensor.matmul(
                                    scps[:, i * QW : (i + 1) * QW],
                                    lhsT=kt_[r0 : r0 + 32, :, kt * PART : (kt + 1) * PART],
                                    rhs=qt[r0 : r0 + 32, :, qc * QW : (qc + 1) * QW],
                                    perf_mode=DR,
                                    tile_position=(r0, 0),
                                )
                            dst = et[:, :, hl, :]
                            if use_dve_exp:
                                src = scps[:].rearrange("p (i q) -> p i q", q=QW)
                                if (n_exp % 16) >= ACT_OF_16:
                                    nc.vector.tensor_scalar(
                                        dst.bitcast(u8),
                                        src,
                                        C0,
                                        60.0,
                                        op0=ALU.mult,
                                        op1=ALU.add,
                                    )
                                else:
                                    nc.scalar.activation(
                                        dst,
                                        src,
                                        AF.Exp,
                                        bias=0.0,
                                        scale=1.0 / 8192.0,
                                    )
                                n_exp += 1
                            else:
                                # general-mask path: per-key-tile bias, ACT only
                                for i in (0, 1):
                                    kt = 2 * j + i
                                    nc.scalar.activation(
                                        et[:, i, hl, :],
                                        scps[:, i * QW : (i + 1) * QW],
                                        AF.Exp,
                                        bias=mb[:, kt : kt + 1],
                                        scale=1.0 / 8192.0,
                                    )
                        for hl in (0, 1):
                            h = 2 * m + hl
                            nc.tensor.matmul(
                                pacc[hl][:],
                                lhsT=v_sb[j][:, :, 68 * h : 68 * h + 65],
                                rhs=et[:, :, hl, :],
                                start=(j == 0),
                                stop=(j == NJ - 1),
                                perf_mode=DR,
                            )
                    for hl in (0, 1):
                        nc.vector.tensor_copy(
                            psb[hl][:, qc * QW : (qc + 1) * QW], pacc[hl][:]
                        )

                # finalize per (m, hl): w = 1/Z via DRAM-bounce reshape,
                # r = sum_q P*w via GpSimd fused multiply+reduce
                for hl in (0, 1):
                    zd = wd_p.tile([1, S], f32, tag="wd", name="zd")
                    nc.sync.dma_start(zd[:], psb[hl][64:65, :])
                    zw = zw_p.tile([PART, KT], f32, tag="zw")
                    nc.sync.dma_start(
                        zw[:], zd[:].rearrange("z (p j) -> (z p) j", p=PART)
                    )
                    nc.vector.reciprocal(zw[:], zw[:])
                    wd = wd_p.tile([1, S], f32, tag="wd", name="wdw")
                    nc.sync.dma_start(
                        wd[:].rearrange("z (p j) -> (z p) j", p=PART), zw[:]
                    )
                    wrep = wrep_p.tile([64, S], f32, tag="wrep")
                    nc.sync.dma_start(wrep[:], wd[:].broadcast_to([64, S]))
                    wtmp = wtmp_p.tile([64, S], f32, tag="wtmp")
                    r = fin_p.tile([64, 1], f32, tag="r")
                    nc.vector.scalar_tensor_tensor(
                        wtmp[:],
                        psb[hl][0:64, :],
                        1.0,
                        wrep[:],
                        op0=ALU.mult,
                        op1=ALU.mult,
                        accum_out=r[:],
                    )
                    nc.sync.dma_start(res[rep, 2 * m + hl, :], r[:])

    nc.compile()
    return nc


def get_nc(repeat: int = 1, use_dve_exp: bool = True):
    key = ("nc", repeat, use_dve_exp)
    if key not in _CACHE:
        _CACHE[key] = _build(repeat, use_dve_exp)
    return _CACHE[key]


def _pair_layout(a2d):
    """[R, C] -> [128, R//128, C] fp8e4m3 pair layout (row r -> partition r%128,
    block r//128)."""
    import ml_dtypes

    r, c = a2d.shape
    return np.ascontiguousarray(
        a2d.reshape(r // PART, PART, c).transpose(1, 0, 2)
    ).astype(ml_dtypes.float8_e4m3)


def make_in_maps(x, mask, Wq, bq, Wk, bk, Wv):
    """Per-core input dict (core c -> batch c//2, head-group c%2)."""
    x = np.asarray(x, np.float32)
    mask = np.asarray(mask)
    maskbias = (mask == 0).astype(np.float32) * NEG  # [B, S]
    xp_b = [_pair_layout(np.ascontiguousarray(x[b].T)) for b in range(B)]
    mbb_b = [np.ascontiguousarray(maskbias[b].reshape(KT, PART).T) for b in range(B)]
    # quadrant permutation of Q/K out-dims: proj chunk c, partition p holds
    # W row for head (c//2)*4 + p//32, d = (c%2)*32 + p%32
    cc, pp = np.meshgrid(np.arange(MT), np.arange(PART), indexing="ij")
    perm = ((cc // 2 * 4 + pp // 32) * 64 + (cc % 2) * 32 + pp % 32).reshape(-1)
    slabs = {}
    for g in range(G):
        sl = slice(g * EG, (g + 1) * EG)
        wq_g = WSCALE * np.asarray(Wq, np.float32)[sl]
        wk_g = WSCALE * np.asarray(Wk, np.float32)[sl]
        slabs[g] = (
            _pair_layout(np.ascontiguousarray(wq_g[perm].T)),
            _pair_layout(np.ascontiguousarray(wk_g[perm].T)),
            _pair_layout(np.ascontiguousarray(WSCALE * np.asarray(Wv, np.float32)[sl].T)),
            np.ascontiguousarray(
                WSCALE * np.asarray(bq, np.float32)[sl][perm].reshape(MT, PART).T
            ),
            np.ascontiguousarray(
                WSCALE * np.asarray(bk, np.float32)[sl][perm].reshape(MT, PART).T
            ),
        )
    in_maps = []
    for c in range(NCORES):
        b, g = c // G, c % G
        wq_t, wk_t, wv_t, bq_t, bk_t = slabs[g]
        in_maps.append(
            {
                "xp": xp_b[b],
                "wqp": wq_t,
                "wkp": wk_t,
                "wvp": wv_t,
                "bq2": bq_t,
                "bk2": bk_t,
                "mbb": mbb_b[b],
            }
        )
    return in_maps


def host_tail(mean_attn, text_array, bv, Wo, bo, W1, b1, W2, b2):
    """Exact tail on [B, E]: out_proj (after the mean), normalize, sub, MLP."""
    out = mean_attn + np.asarray(bv, np.float32)[None, :]
    out = out @ np.asarray(Wo, np.float32).T + np.asarray(bo, np.float32)
    out = out / np.linalg.norm(out, axis=-1, keepdims=True)
    out = out - np.asarray(text_array, np.float32)
    h = np.maximum(out @ np.asarray(W1, np.float32).T + np.asarray(b1, np.float32), 0.0)
    return np.tanh(h @ np.asarray(W2, np.float32).T + np.asarray(b2, np.float32))


def kernel(
    x, mask, text_array, Wq, bq, Wk, bk, Wv, bv, Wo, bo, W1, b1, W2, b2
):
    from concourse.bass_utils import run_bass_kernel_spmd

    use_dve = bool((np.asarray(mask) != 0).all())
    nc = get_nc(use_dve_exp=use_dve)
    in_maps = make_in_maps(x, mask, Wq, bq, Wk, bk, Wv)
    out = run_bass_kernel_spmd(nc, in_maps, core_ids=list(range(NCORES)))
    mean_attn = np.zeros((B, E), np.float32)
    for c in range(NCORES):
        b, g = c // G, c % G
        r = out.results[c]["res"][0]  # [HG, D] = sum_q attn_out * 32; fix below
        mean_attn[b, g * EG : (g + 1) * EG] = r.reshape(EG) / (WSCALE * S)
    return host_tail(mean_attn, text_array, bv, Wo, bo, W1, b1, W2, b2).astype(
        np.float32
    )


# revision 16
# speedup vs baseline: 1.7696x; 1.3360x over previous
"""Trainium2 Bass kernel for nn_MultiHeadAttention_446676599023 (v2, fp8).

Strategy (8 NeuronCores, SPMD, no collectives):
  core c -> batch b = c//2, head-group g = c%2 (heads 8g..8g+7, E-dims 512g..512g+512).

The mean over S commutes with the output projection, so each core returns only
  r_h[d] = sum_q softmax_row(q) @ V_h   (shape [64] per head)
and the Wo/normalize/MLP tail runs on the host on a [4,1024] tensor (exact algebra).

Device pipeline per core (all matmul operands fp8):
  - Projections in fp8e4m3 with DoubleRow (contraction pairs packed along the
    free dim): Q/K -> [128, S] per head-pair (2 heads on PE row-groups 0/64),
    V -> [keys, e'] with a per-head ones column for the softmax denominator.
    Weights are pre-scaled by 32 on the host so fp8 sees ~N(0,1) values.
  - Scores: fp8e4m3 DoubleRow matmuls with K=32 contraction pairs. Each head
    owns one 32-partition quadrant of two [128, 2, S] Q/K tiles (host permutes
    W rows so the projection lands directly in this layout); tile_position
    places the matmul in the head's PE quadrant. s_psum = 1024 * (q . k), f32.
    (Plain fp8 K=64 matmuls packed on row-groups 0/64 hang the device - FWL
    weight loads racing the other row-group's in-flight matmul; DoubleRow
    disables FWL, so this path is also the workaround.)
  - exp: split between ScalarE (true exp, output fp8e5m2) and VectorE using a
    Schraudolph bit-trick: u8 = s_psum*(log2e/2048) + 60.5 truncated to uint8
    IS the fp8e5m2 encoding of exp(s_psum/8192) (log-linear mantissa approx,
    ~5% per-element error that cancels in softmax normalization).
  - E@V: fp8 DoubleRow over key-tile pairs (lhsT = V_aug pairs [128,2,65],
    rhs = E pairs [128,2,512]), accumulated in PSUM; row 64 is the softmax
    denominator Z via the ones column.
  - finalize: Z -> DRAM bounce -> [128,16] -> reciprocal (DVE) -> DRAM ->
    broadcast wrep [64,2048]; GpSimd fused multiply+reduce gives
    r = sum_q P[:,q] * w[q]. Host divides by 32*S and applies the tail.
"""

import math
import os
import sys

import numpy as np

for _p in ("/opt/trn_rl_repo",):
    if _p not in sys.path and os.path.isdir(_p):
        sys.path.append(_p)

B, S, E, H = 4, 2048, 1024, 16
D = E // H            # 64 head dim
G = 2                 # head groups (tensor-parallel factor)
EG = E // G           # 512 dims per group
HG = H // G           # 8 heads per group
NCORES = 8
PART = 128
ET = E // PART        # 8 contraction tiles
NPAIR = ET // 2       # 4 contraction pairs for DoubleRow
KT = S // PART        # 16 key tiles
NJ = KT // 2          # 8 key-tile pairs
MT = EG // PART       # 4 m-tiles (= head pairs) per group
QC = 4                # q chunks
QW = S // QC          # 512
NEG = -1.0e30
WSCALE = 32.0         # host-side weight prescale for fp8
LOG2E = 1.4426950408889634
# of every 16 exp tiles, this many go to ScalarE (rest: DVE bit-trick)
ACT_OF_16 = 10

_CACHE: dict = {}


def _build(repeat: int = 1, use_dve_exp: bool = True):
    import concourse.bacc as bacc
    import concourse.mybir as mybir
    import concourse.tile as tile
    from contextlib import ExitStack

    f32 = mybir.dt.float32
    f8e4 = mybir.dt.float8e4
    f8e5 = mybir.dt.float8e5
    u8 = mybir.dt.uint8
    AF = mybir.ActivationFunctionType
    ALU = mybir.AluOpType
    DR = mybir.MatmulPerfMode.DoubleRow

    nc = bacc.Bacc("TRN2", target_bir_lowering=False, debug=False)
    xp = nc.dram_tensor("xp", [PART, ET, S], f8e4, kind="ExternalInput").ap()
    wqp = nc.dram_tensor("wqp", [PART, ET, EG], f8e4, kind="ExternalInput").ap()
    wkp = nc.dram_tensor("wkp", [PART, ET, EG], f8e4, kind="ExternalInput").ap()
    wvp = nc.dram_tensor("wvp", [PART, ET, EG], f8e4, kind="ExternalInput").ap()
    bq2 = nc.dram_tensor("bq2", [PART, MT], f32, kind="ExternalInput").ap()
    bk2 = nc.dram_tensor("bk2", [PART, MT], f32, kind="ExternalInput").ap()
    mbb = nc.dram_tensor("mbb", [PART, KT], f32, kind="ExternalInput").ap()
    res = nc.dram_tensor("res", [repeat, HG, D], f32, kind="ExternalOutput").ap()

    C0 = LOG2E / 2048.0  # DVE trick scale on s_psum

    with tile.TileContext(nc) as tc, ExitStack() as ctx:
        const_p = ctx.enter_context(tc.tile_pool(name="const", bufs=1))
        xp_p = ctx.enter_context(tc.tile_pool(name="xp", bufs=1))
        w_p = ctx.enter_context(tc.tile_pool(name="w", bufs=3))
        qk_p = ctx.enter_context(tc.tile_pool(name="qk", bufs=2 * MT))
        v_p = ctx.enter_context(tc.tile_pool(name="v", bufs=NJ))
        e_p = ctx.enter_context(tc.tile_pool(name="e", bufs=3))
        psb_p = ctx.enter_context(tc.tile_pool(name="psb", bufs=4))
        zw_p = ctx.enter_context(tc.tile_pool(name="zw", bufs=2))
        wrep_p = ctx.enter_context(tc.tile_pool(name="wrep", bufs=2))
        wtmp_p = ctx.enter_context(tc.tile_pool(name="wtmp", bufs=2))
        fin_p = ctx.enter_context(tc.tile_pool(name="fin", bufs=4))
        # PSUM: scores 3 x [128,1024] (2 banks each) + pacc 2 x [65,512]
        sc_ps = ctx.enter_context(tc.tile_pool(name="scps", bufs=3, space="PSUM"))
        p_ps = ctx.enter_context(tc.tile_pool(name="pps", bufs=2, space="PSUM"))
        wd_p = ctx.enter_context(tc.tile_pool(name="wd", bufs=4, space="DRAM"))

        for rep in range(repeat):
            mb = const_p.tile([PART, KT], f32, tag="mb")
            nc.sync.dma_start(mb[:], mbb[:])
            bq_t = const_p.tile([PART, MT], f32, tag="bq")
            nc.sync.dma_start(bq_t[:], bq2[:])
            bk_t = const_p.tile([PART, MT], f32, tag="bk")
            nc.sync.dma_start(bk_t[:], bk2[:])

            x_t = xp_p.tile([PART, ET, S], f8e4, tag="x")
            nc.sync.dma_start(x_t[:], xp[:])
            wq_t = w_p.tile([PART, ET, EG], f8e4, tag="w", name="wq")
            nc.sync.dma_start(wq_t[:], wqp[:])
            wk_t = w_p.tile([PART, ET, EG], f8e4, tag="w", name="wk")
            nc.sync.dma_start(wk_t[:], wkp[:])
            wv_t = w_p.tile([PART, ET, EG], f8e4, tag="w", name="wv")
            nc.sync.dma_start(wv_t[:], wvp[:])

            # ---------------- projections (fp8 DoubleRow) ----------------
            # Q/K land in quadrant layout: tile t in {0,1} is [128, 2, S];
            # head h = 4t + p//32 lives on partitions 32(h%4):32(h%4)+32 with
            # d = i*32 + p%32 (host permutes W rows/bias to match).
            qk = {}
            n_conv = 0
            for which in ("q", "k"):
                for t_ in range(2):
                    qk[(which, t_)] = qk_p.tile(
                        [PART, 2, S], f8e4, tag="qk", name=f"{which}{t_}"
                    )
            conv_n = [0]

            def emit_qk_proj(c):
                t_, i_ = c // 2, c % 2
                for which, w_t, b_t in (("q", wq_t, bq_t), ("k", wk_t, bk_t)):
                    t = qk[(which, t_)]
                    for qc in range(QC):
                        ps = sc_ps.tile([PART, 2 * QW], f32, tag="sc", name="qkps")
                        for i in range(NPAIR):
                            nc.tensor.matmul(
                                ps[:, 0:QW],
                                lhsT=w_t[:, 2 * i : 2 * i + 2, c * PART : (c + 1) * PART],
                                rhs=x_t[:, 2 * i : 2 * i + 2, qc * QW : (qc + 1) * QW],
                                start=(i == 0),
                                stop=(i == NPAIR - 1),
                                perf_mode=DR,
                            )
                        # psum -> fp8 with bias, alternating ACT/DVE
                        if conv_n[0] % 2 == 0:
                            nc.vector.tensor_scalar_add(
                                t[:, i_, qc * QW : (qc + 1) * QW],
                                ps[:, 0:QW],
                                b_t[:, c : c + 1],
                            )
                        else:
                            nc.scalar.add(
                                t[:, i_, qc * QW : (qc + 1) * QW],
                                ps[:, 0:QW],
                                b_t[:, c : c + 1],
                            )
                        conv_n[0] += 1

            # tile 0 (heads 0-3) now; tile 1 (heads 4-7) mid-attention
            for c in (0, 1):
                emit_qk_proj(c)

            # ---------------- V projection (fp8 DoubleRow) ----------------
            v_sb = []
            for j in range(NJ):
                vt = v_p.tile([PART, 2, HG * 68], f8e4, tag="v")
                v4 = vt[:].rearrange("p i (h c) -> p i h c", c=68)
                nc.gpsimd.memset(v4[:, :, :, 64:68], 1.0)
                v_sb.append(vt)
            for ks in range(KT):
                ps = sc_ps.tile([PART, 2 * QW], f32, tag="sc", name="vps")
                for i in range(NPAIR):
                    nc.tensor.matmul(
                        ps[:, 0:QW],
                        lhsT=x_t[:, 2 * i : 2 * i + 2, ks * PART : (ks + 1) * PART],
                        rhs=wv_t[:, 2 * i : 2 * i + 2, :],
                        start=(i == 0),
                        stop=(i == NPAIR - 1),
                        perf_mode=DR,
                    )
                v4 = v_sb[ks // 2][:].rearrange("p i (h c) -> p i h c", c=68)
                dst = v4[:, ks % 2, :, 0:64]
                src = ps[:, 0:QW].rearrange("p (h c) -> p h c", c=64)
                if ks % 2 == 0:
                    nc.vector.tensor_copy(dst, src)
                else:
                    nc.scalar.copy(dst, src)

            # ---------------- attention ----------------
            # Software-pipelined: scores+exp for step j are emitted before the
            # E@V of step j-1, so the in-order PE queue never stalls on exp.
            n_exp = 0
            for m in range(MT):
                psb = {}
                for hl in (0, 1):
                    psb[hl] = psb_p.tile([65, S], f32, tag="psb", name=f"psb{m}_{hl}")
                for qc in range(QC):
                    pacc = {}
                    for hl in (0, 1):
                        pacc[hl] = p_ps.tile([65, QW], f32, tag="pp", name=f"pacc{hl}")
                    ets = {}
                    for j in range(NJ + 1):
                        if j < NJ:
                            et = e_p.tile([PART, 2, 2, QW], f8e5, tag="e")
                            ets[j] = et
                            scps = {}
                            for hl in (0, 1):
                                scps[hl] = sc_ps.tile(
                                    [PART, 2 * QW], f32, tag="sc", name="scps"
                                )
                            # kt-parity outer, head inner: adjacent matmuls hit
                            # different PE quadrants and overlap
                            for i in (0, 1):
                                kt = 2 * j + i
                                for hl in (0, 1):
                                    h = 2 * m + hl
                                    t_, qd = h // 4, h % 4
                                    r0 = 32 * qd
                                    nc.tensor.matmul(
                                        scps[hl][:, i * QW : (i + 1) * QW],
                                        lhsT=qk[("k", t_)][
                                            r0 : r0 + 32, :, kt * PART : (kt + 1) * PART
                                        ],
                                        rhs=qk[("q", t_)][
                                            r0 : r0 + 32, :, qc * QW : (qc + 1) * QW
                                        ],
                                        perf_mode=DR,
                                        tile_position=(r0, 0),
                                    )
                            for hl in (0, 1):
                                dst = et[:, :, hl, :]
                                if use_dve_exp:
                                    src = scps[hl][:].rearrange(
                                        "p (i q) -> p i q", q=QW
                                    )
                                    if (n_exp % 16) >= ACT_OF_16:
                                        nc.vector.tensor_scalar(
                                            dst.bitcast(u8),
                                            src,
                                            C0,
                                            60.0,
                                            op0=ALU.mult,
                                            op1=ALU.add,
                                        )
                                    else:
                                        nc.scalar.activation(
                                            dst,
                                            src,
                                            AF.Exp,
                                            bias=0.0,
                                            scale=1.0 / 8192.0,
                                        )
                                    n_exp += 1
                                else:
                                    # general-mask path: per-key-tile bias
                                    for i in (0, 1):
                                        kt = 2 * j + i
                                        nc.scalar.activation(
                                            et[:, i, hl, :],
                                            scps[hl][:, i * QW : (i + 1) * QW],
                                            AF.Exp,
                                            bias=mb[:, kt : kt + 1],
                                            scale=1.0 / 8192.0,
                                        )
                        if j >= 1:
                            jj = j - 1
                            for hl in (0, 1):
                                h = 2 * m + hl
                                nc.tensor.matmul(
                                    pacc[hl][:],
                                    lhsT=v_sb[jj][:, :, 68 * h : 68 * h + 65],
                                    rhs=ets[jj][:, :, hl, :],
                                    start=(jj == 0),
                                    stop=(jj == NJ - 1),
                                    perf_mode=DR,
                                )
                    for hl in (0, 1):
                        nc.vector.tensor_copy(
                            psb[hl][:, qc * QW : (qc + 1) * QW], pacc[hl][:]
                        )

                # finalize per (m, hl): w = 1/Z via DRAM-bounce reshape,
                # r = sum_q P*w via GpSimd fused multiply+reduce
                for hl in (0, 1):
                    zd = wd_p.tile([1, S], f32, tag="wd", name="zd")
                    nc.sync.dma_start(zd[:], psb[hl][64:65, :])
                    zw = zw_p.tile([PART, KT], f32, tag="zw")
                    nc.sync.dma_start(
                        zw[:], zd[:].rearrange("z (p j) -> (z p) j", p=PART)
                    )
                    nc.vector.reciprocal(zw[:], zw[:])
                    wd = wd_p.tile([1, S], f32, tag="wd", name="wdw")
                    nc.sync.dma_start(
                        wd[:].rearrange("z (p j) -> (z p) j", p=PART), zw[:]
                    )
                    wrep = wrep_p.tile([64, S], f32, tag="wrep")
                    nc.sync.dma_start(wrep[:], wd[:].broadcast_to([64, S]))
                    wtmp = wtmp_p.tile([64, S], f32, tag="wtmp")
                    r = fin_p.tile([64, 1], f32, tag="r")
                    nc.vector.scalar_tensor_tensor(
                        wtmp[:],
                        psb[hl][0:64, :],
                        1.0,
                        wrep[:],
                        op0=ALU.mult,
                        op1=ALU.mult,
                        accum_out=r[:],
                    )
                    nc.sync.dma_start(res[rep, 2 * m + hl, :], r[:])

                if m == 1:
                    # heads 4-7's Q/K projections, overlapped with attention
                    for c in (2, 3):
                        emit_qk_proj(c)

    nc.compile()
    return nc


def get_nc(repeat: int = 1, use_dve_exp: bool = True):
    key = ("nc", repeat, use_dve_exp)
    if key not in _CACHE:
        _CACHE[key] = _build(repeat, use_dve_exp)
    return _CACHE[key]


def _pair_layout(a2d):
    """[R, C] -> [128, R//128, C] fp8e4m3 pair layout (row r -> partition r%128,
    block r//128)."""
    import ml_dtypes

    r, c = a2d.shape
    return np.ascontiguousarray(
        a2d.reshape(r // PART, PART, c).transpose(1, 0, 2)
    ).astype(ml_dtypes.float8_e4m3)


def make_in_maps(x, mask, Wq, bq, Wk, bk, Wv):
    """Per-core input dict (core c -> batch c//2, head-group c%2)."""
    x = np.asarray(x, np.float32)
    mask = np.asarray(mask)
    maskbias = (mask == 0).astype(np.float32) * NEG  # [B, S]
    xp_b = [_pair_layout(np.ascontiguousarray(x[b].T)) for b in range(B)]
    mbb_b = [np.ascontiguousarray(maskbias[b].reshape(KT, PART).T) for b in range(B)]
    # quadrant permutation of Q/K out-dims: proj chunk c, partition p holds
    # W row for head (c//2)*4 + p//32, d = (c%2)*32 + p%32
    cc, pp = np.meshgrid(np.arange(MT), np.arange(PART), indexing="ij")
    perm = ((cc // 2 * 4 + pp // 32) * 64 + (cc % 2) * 32 + pp % 32).reshape(-1)
    slabs = {}
    for g in range(G):
        sl = slice(g * EG, (g + 1) * EG)
        wq_g = WSCALE * np.asarray(Wq, np.float32)[sl]
        wk_g = WSCALE * np.asarray(Wk, np.float32)[sl]
        slabs[g] = (
            _pair_layout(np.ascontiguousarray(wq_g[perm].T)),
            _pair_layout(np.ascontiguousarray(wk_g[perm].T)),
            _pair_layout(np.ascontiguousarray(WSCALE * np.asarray(Wv, np.float32)[sl].T)),
            np.ascontiguousarray(
                WSCALE * np.asarray(bq, np.float32)[sl][perm].reshape(MT, PART).T
            ),
            np.ascontiguousarray(
                WSCALE * np.asarray(bk, np.float32)[sl][perm].reshape(MT, PART).T
            ),
        )
    in_maps = []
    for c in range(NCORES):
        b, g = c // G, c % G
        wq_t, wk_t, wv_t, bq_t, bk_t = slabs[g]
        in_maps.append(
            {
                "xp": xp_b[b],
                "wqp": wq_t,
                "wkp": wk_t,
                "wvp": wv_t,
                "bq2": bq_t,
                "bk2": bk_t,
                "mbb": mbb_b[b],
            }
        )
    return in_maps


def host_tail(mean_attn, text_array, bv, Wo, bo, W1, b1, W2, b2):
    """Exact tail on [B, E]: out_proj (after the mean), normalize, sub, MLP."""
    out = mean_attn + np.asarray(bv, np.float32)[None, :]
    out = out @ np.asarray(Wo, np.float32).T + np.asarray(bo, np.float32)
    out = out / np.linalg.norm(out, axis=-1, keepdims=True)
    out = out - np.asarray(text_array, np.float32)
    h = np.maximum(out @ np.asarray(W1, np.float32).T + np.asarray(b1, np.float32), 0.0)
    return np.tanh(h @ np.asarray(W2, np.float32).T + np.asarray(b2, np.float32))


def kernel(
    x, mask, text_array, Wq, bq, Wk, bk, Wv, bv, Wo, bo, W1, b1, W2, b2
):
    from concourse.bass_utils import run_bass_kernel_spmd

    use_dve = bool((np.asarray(mask) != 0).all())
    nc = get_nc(use_dve_exp=use_dve)
    in_maps = make_in_maps(x, mask, Wq, bq, Wk, bk, Wv)
    out = run_bass_kernel_spmd(nc, in_maps, core_ids=list(range(NCORES)))
    mean_attn = np.zeros((B, E), np.float32)
    for c in range(NCORES):
        b, g = c // G, c % G
        r = out.results[c]["res"][0]  # [HG, D] = sum_q attn_out * 32; fix below
        mean_attn[b, g * EG : (g + 1) * EG] = r.reshape(EG) / (WSCALE * S)
    return host_tail(mean_attn, text_array, bv, Wo, bo, W1, b1, W2, b2).astype(
        np.float32
    )
